# revision 3
# baseline (speedup 1.0000x reference)
"""Trainium2 Bass kernel for nn_CanadarmJacob (space-arm Jacobian, bm=1 path).

Contract: kernel(**inputs) takes FULL inputs (com_list (512,256,3,7) f32,
link_pose_list (512,256,4,4,9) f32, bm scalar) and returns the FULL output
(512,256,6,7) f32. Internally shards samples across 8 NeuronCores (pure data
parallel), runs a Bass/Tile kernel per core, and gathers.

Design (v3): fp16 datapath (DVE 2x_1p mode on tensor_tensor), plane layout
(P=128 partitions x comp x J=128 samples, J innermost/contiguous). Mass and
inertia constants pre-scaled by SC=1/64 so intermediates stay inside fp16
range; the scale cancels in bot = A @ Htheta because A inverts the scaled
H_s. Engine split: DVE runs the big fp16 streams, GpSimd runs the per-sample
fp32 3x3-inverse chain (J-length rows) plus the per-act constant scalings,
ScalarE (ACT) runs off-critical-path copies and dtype casts. Inputs are three
DMA tensors ordered so the first compute waits only on pos+com.
"""
import sys
import functools

if "/opt/trn_rl_repo" not in sys.path:
    sys.path.insert(0, "/opt/trn_rl_repo")

import numpy as np

# ---------------------------------------------------------------- constants
N_CORES = 8
P = 128          # SBUF partitions
J = 128          # samples per partition per core
N_ACT = 7
SC = 1.0 / 64.0  # range scale for fp16

MASS = np.array([105.98, 105.98, 314.98, 279.2, 105.98, 105.98, 243.66], np.float64)
TM = float(MASS.sum() + 100000.0 + 243.66)
DIAGS = np.array([[12.19, 12.19, 3.061], [12.19, 12.19, 3.061], [15.41, 2094.71, 2103.19],
                  [9.522, 1966.28, 1966.28], [8.305, 3.061, 8.0386], [12.13, 12.13, 3.061],
                  [9.336, 44.41, 44.41]], np.float64)
D_SUF = np.cumsum(DIAGS[::-1], axis=0)[::-1]          # (7,3) suffix inertia diag
SM = np.cumsum(MASS[::-1])[::-1]                      # (7,) suffix mass
CD = DIAGS.sum(axis=0)                                # (3,)
_TF0 = np.array([[1, 0, 0, 0], [0, -1, 0, 0], [0, 0, 1.3, 6], [0, 0, 0, 1]], np.float64)
_COM0 = np.array([[1, 0, 0, 0], [0, 1, 0, 0], [0, 0, 1, 0.5], [0, 0, 0, 1]], np.float64)
BASE = (_TF0 @ _COM0)[:3, 3] * 243.66 / (100000.0 + 243.66)   # [0, 0, ~0.0162]

TMS = TM * SC
CDS = CD * SC
MS = MASS * SC
AXIS = np.array([2, 0, 2, 2, 2, 0, 2])
LINK = np.arange(N_ACT)
SIGN = np.array([1., 1., 1., 1., -1., 1., 1.], np.float32)


def _emit(nc, tc, ctx, dram):
    from concourse import mybir

    f16 = mybir.dt.float16
    f32 = mybir.dt.float32
    OP = mybir.AluOpType
    V = nc.vector
    G = nc.gpsimd
    S = nc.scalar

    pool = ctx.enter_context(tc.tile_pool(name="main", bufs=1))

    # fp16 tiles; *E tiles carry c-plane wrap-around extension [x,y,z,x,y]
    pc = pool.tile([P, 6, 7, J], f16)       # pos(0:3) | com(3:6)
    rot = pool.tile([P, 5, 7, J], f16)
    aux = pool.tile([P, 5, 7, J], f16)      # drot(0:3) | SM*SC plane | -SM/TM plane
    delE = pool.tile([P, 5, 7, J], f16)
    mdel = pool.tile([P, 3, 7, J], f16)     # becomes w after suffix cumsum
    scr1 = pool.tile([P, 3, 7, J], f16)
    scr2 = pool.tile([P, 3, 7, J], f16)
    jacE = pool.tile([P, 5, 7, J], f16)
    prod = pool.tile([P, 9, 7, J], f16)     # Sdiag | Soff | mcom products
    tscr = pool.tile([P, 9, 3, J], f16)
    red16 = pool.tile([P, 9, J], f16)
    vscr = pool.tile([P, 9, J], f16)
    smc = pool.tile([P, 3, 7, J], f16)
    w2E = pool.tile([P, 5, 7, J], f16)
    jsm = pool.tile([P, 3, 7, J], f16)
    hthE = pool.tile([P, 5, 7, J], f16)
    outE = pool.tile([P, 8, 7, J], f16)     # top(0:3) | bot(3:6) | bot-ext(6:8)
    c16 = pool.tile([P, 5, J], f16)
    A16 = pool.tile([P, 8, J], f16)         # a11,a22,a33,a12,a23,a13,a12,a23
    # fp32 smalls (GpSimd)
    red32 = pool.tile([P, 9, J], f32)       # Sd(xx,yy,zz) | So(xy,yz,zx) | scom
    c32 = pool.tile([P, 5, J], f32)
    sm32 = pool.tile([P, 8, J], f32)
    hs = pool.tile([P, 6, J], f32)          # hxx,hyy,hzz,hxy,hyz,hzx
    adj = pool.tile([P, 6, J], f32)         # a11,a22,a33,a12,a23,a13
    A32 = pool.tile([P, 6, J], f32)

    nc.sync.dma_start(out=pc[:], in_=dram["pc"][:])
    nc.sync.dma_start(out=rot[:, 0:3], in_=dram["rot3"][:])
    nc.sync.dma_start(out=aux[:], in_=dram["aux"][:])

    def bc_c(ap):   # (P,7,J) -> (P,3,7,J), broadcast over coord planes
        return ap.unsqueeze(1).broadcast_to((P, 3, 7, J))

    def bc_a(ap, jn=J):   # (P,3,jn) -> (P,3,7,jn), broadcast over act
        return ap.unsqueeze(2).broadcast_to((P, 3, 7, jn))

    # --- del; const scalings on GpSimd ----------------------------------
    S.copy(out=rot[:, 3:5], in_=rot[:, 0:2])
    V.tensor_tensor(out=delE[:, 0:3], in0=pc[:, 3:6], in1=pc[:, 0:3], op=OP.subtract)
    S.copy(out=delE[:, 3:5], in_=delE[:, 0:2])
    for a in range(N_ACT):
        G.tensor_scalar(out=mdel[:, :, a], in0=delE[:, 0:3, a],
                        scalar1=float(MS[a]), scalar2=None, op0=OP.mult)
    for a in range(N_ACT):
        G.tensor_scalar(out=prod[:, 6:9, a], in0=pc[:, 3:6, a],
                        scalar1=float(MS[a]), scalar2=None, op0=OP.mult)

    # --- jac cross -------------------------------------------------------
    V.tensor_tensor(out=scr1[:], in0=rot[:, 1:4], in1=delE[:, 2:5], op=OP.mult)
    V.tensor_tensor(out=scr2[:], in0=rot[:, 2:5], in1=delE[:, 1:4], op=OP.mult)
    V.tensor_tensor(out=jacE[:, 0:3], in0=scr1[:], in1=scr2[:], op=OP.subtract)
    S.copy(out=jacE[:, 3:5], in_=jacE[:, 0:2])

    # --- S products + act-reduction tree ---------------------------------
    V.tensor_tensor(out=prod[:, 0:3], in0=mdel[:], in1=delE[:, 0:3], op=OP.mult)
    V.tensor_tensor(out=prod[:, 3:6], in0=mdel[:], in1=delE[:, 1:4], op=OP.mult)
    V.tensor_tensor(out=tscr[:], in0=prod[:, :, 0:3], in1=prod[:, :, 3:6], op=OP.add)
    V.tensor_tensor(out=red16[:], in0=tscr[:, :, 0], in1=tscr[:, :, 1], op=OP.add)
    V.tensor_tensor(out=vscr[:], in0=tscr[:, :, 2], in1=prod[:, :, 6], op=OP.add)
    V.tensor_tensor(out=red16[:], in0=red16[:], in1=vscr[:], op=OP.add)
    S.copy(out=red32[:], in_=red16[:])

    # --- fp32 smalls on GpSimd: c, H_s, adjugate, det --------------------
    inv_tms = 1.0 / TMS
    G.tensor_scalar(out=c32[:, 0:2], in0=red32[:, 6:8], scalar1=inv_tms,
                    scalar2=None, op0=OP.mult)
    G.tensor_scalar(out=c32[:, 2], in0=red32[:, 8], scalar1=inv_tms,
                    scalar2=float(BASE[2]), op0=OP.mult, op1=OP.subtract)
    G.tensor_scalar(out=c32[:, 3:5], in0=c32[:, 0:2], scalar1=1.0,
                    scalar2=None, op0=OP.mult)
    S.copy(out=c16[:], in_=c32[:])

    # sm32 rows: 0=SS 1=q 2=csq 3..7 scratch
    G.tensor_tensor(out=sm32[:, 0], in0=red32[:, 0], in1=red32[:, 1], op=OP.add)
    G.tensor_tensor(out=sm32[:, 0], in0=sm32[:, 0], in1=red32[:, 2], op=OP.add)
    ccd = sm32[:, 5:8]
    G.tensor_tensor(out=ccd, in0=c32[:, 0:3], in1=c32[:, 0:3], op=OP.mult)
    G.tensor_tensor(out=sm32[:, 2], in0=sm32[:, 5], in1=sm32[:, 6], op=OP.add)
    G.tensor_tensor(out=sm32[:, 2], in0=sm32[:, 2], in1=sm32[:, 7], op=OP.add)
    # q = SS - TMS*csq
    G.tensor_scalar(out=sm32[:, 1], in0=sm32[:, 2], scalar1=TMS, scalar2=None,
                    op0=OP.mult)
    G.tensor_tensor(out=sm32[:, 1], in0=sm32[:, 0], in1=sm32[:, 1], op=OP.subtract)
    # hs diag: TMS*cc_d + CD_c, then + (q - Sd)
    for cx in range(3):
        G.tensor_scalar(out=hs[:, cx], in0=sm32[:, 5 + cx], scalar1=TMS,
                        scalar2=float(CDS[cx]), op0=OP.mult, op1=OP.add)
    qb = sm32[:, 1].unsqueeze(1).broadcast_to((P, 3, J))
    G.tensor_tensor(out=sm32[:, 3:6], in0=qb, in1=red32[:, 0:3], op=OP.subtract)
    G.tensor_tensor(out=hs[:, 0:3], in0=hs[:, 0:3], in1=sm32[:, 3:6], op=OP.add)
    # hs off: TMS*cc_o - So  (cc_o = c * c[[y,z,x]])
    G.tensor_tensor(out=sm32[:, 3:6], in0=c32[:, 0:3], in1=c32[:, 1:4], op=OP.mult)
    G.tensor_scalar(out=sm32[:, 3:6], in0=sm32[:, 3:6], scalar1=TMS,
                    scalar2=None, op0=OP.mult)
    G.tensor_tensor(out=hs[:, 3:6], in0=sm32[:, 3:6], in1=red32[:, 3:6], op=OP.subtract)

    # adjugate: adj rows [a11,a22,a33,a12,a23,a13]
    h2b = hs[:, 2].unsqueeze(1).broadcast_to((P, 2, J))
    h3b = hs[:, 3].unsqueeze(1).broadcast_to((P, 2, J))
    G.tensor_tensor(out=sm32[:, 3:5], in0=hs[:, 1::-1], in1=h2b, op=OP.mult)
    G.tensor_tensor(out=sm32[:, 5:7], in0=hs[:, 4:6], in1=hs[:, 4:6], op=OP.mult)
    G.tensor_tensor(out=adj[:, 0:2], in0=sm32[:, 3:5], in1=sm32[:, 5:7], op=OP.subtract)
    G.tensor_tensor(out=sm32[:, 3], in0=hs[:, 0], in1=hs[:, 1], op=OP.mult)
    G.tensor_tensor(out=sm32[:, 4], in0=hs[:, 3], in1=hs[:, 3], op=OP.mult)
    G.tensor_tensor(out=adj[:, 2], in0=sm32[:, 3], in1=sm32[:, 4], op=OP.subtract)
    # (a23,a13) = h3*[h5,h4] - [h0,h1]*[h4,h5]
    G.tensor_tensor(out=sm32[:, 3:5], in0=h3b, in1=hs[:, 5:3:-1], op=OP.mult)
    G.tensor_tensor(out=sm32[:, 5:7], in0=hs[:, 0:2], in1=hs[:, 4:6], op=OP.mult)
    G.tensor_tensor(out=adj[:, 4:6], in0=sm32[:, 3:5], in1=sm32[:, 5:7], op=OP.subtract)
    # a12 = h4h5 - h3h2
    G.tensor_tensor(out=sm32[:, 3], in0=hs[:, 4], in1=hs[:, 5], op=OP.mult)
    G.tensor_tensor(out=sm32[:, 4], in0=hs[:, 3], in1=hs[:, 2], op=OP.mult)
    G.tensor_tensor(out=adj[:, 3], in0=sm32[:, 3], in1=sm32[:, 4], op=OP.subtract)
    # det = h0*a11 + h3*a12 + h5*a13
    G.tensor_tensor(out=sm32[:, 3], in0=hs[:, 0], in1=adj[:, 0], op=OP.mult)
    G.tensor_tensor(out=sm32[:, 4], in0=hs[:, 3], in1=adj[:, 3], op=OP.mult)
    G.tensor_tensor(out=sm32[:, 5], in0=hs[:, 5], in1=adj[:, 5], op=OP.mult)
    G.tensor_tensor(out=sm32[:, 6], in0=sm32[:, 3], in1=sm32[:, 4], op=OP.add)
    G.tensor_tensor(out=sm32[:, 6], in0=sm32[:, 6], in1=sm32[:, 5], op=OP.add)

    # --- w = suffix cumsum(mdel); w2 = w - SM*c --------------------------
    for k in range(5, -1, -1):
        V.tensor_tensor(out=mdel[:, :, k], in0=mdel[:, :, k], in1=mdel[:, :, k + 1],
                        op=OP.add)
    V.tensor_tensor(out=smc[:], in0=bc_c(aux[:, 3]), in1=bc_a(c16[:, 0:3]), op=OP.mult)
    V.tensor_tensor(out=w2E[:, 0:3], in0=mdel[:], in1=smc[:], op=OP.subtract)
    V.tensor_copy(out=w2E[:, 3:5], in_=w2E[:, 0:2])

    # --- jsm, Htheta -----------------------------------------------------
    V.tensor_tensor(out=jsm[:], in0=bc_c(aux[:, 4]), in1=jacE[:, 0:3], op=OP.mult)
    V.tensor_tensor(out=scr1[:], in0=w2E[:, 1:4], in1=jacE[:, 2:5], op=OP.mult)
    V.tensor_tensor(out=scr2[:], in0=w2E[:, 2:5], in1=jacE[:, 1:4], op=OP.mult)
    V.tensor_tensor(out=hthE[:, 0:3], in0=scr1[:], in1=scr2[:], op=OP.subtract)
    V.tensor_tensor(out=hthE[:, 0:3], in0=hthE[:, 0:3], in1=aux[:, 0:3], op=OP.add)
    V.tensor_copy(out=hthE[:, 3:5], in_=hthE[:, 0:2])

    # reciprocal of det on DVE (queued late so det is long ready); A on GpSimd
    V.reciprocal(out=sm32[:, 7], in_=sm32[:, 6])
    G.tensor_scalar(out=sm32[:, 7], in0=sm32[:, 7], scalar1=-1.0, scalar2=None,
                    op0=OP.mult)
    rdb = sm32[:, 7].unsqueeze(1).broadcast_to((P, 6, J))
    G.tensor_tensor(out=A32[:], in0=adj[:], in1=rdb, op=OP.mult)
    S.copy(out=A16[:, 0:6], in_=A32[:])
    S.copy(out=A16[:, 6:8], in_=A16[:, 3:5])

    # --- bot = A @ Hth ---------------------------------------------------
    def Ab(r):
        return A16[:, r:r + 3].unsqueeze(2).broadcast_to((P, 3, 7, J))

    V.tensor_tensor(out=scr1[:], in0=Ab(0), in1=hthE[:, 0:3], op=OP.mult)
    V.tensor_tensor(out=scr2[:], in0=Ab(3), in1=hthE[:, 1:4], op=OP.mult)
    V.tensor_tensor(out=scr1[:], in0=scr1[:], in1=scr2[:], op=OP.add)
    V.tensor_tensor(out=scr2[:], in0=Ab(5), in1=hthE[:, 2:5], op=OP.mult)
    V.tensor_tensor(out=outE[:, 3:6], in0=scr1[:], in1=scr2[:], op=OP.add)
    V.tensor_copy(out=outE[:, 6:8], in_=outE[:, 3:5])
    nc.sync.dma_start(out=dram["out"][:, 3:6], in_=outE[:, 3:6])

    # --- top = c x bot + jsm, in J-halves to overlap the output DMA ------
    JH = J // 2
    for h in range(2):
        js = slice(h * JH, (h + 1) * JH)
        c1b = c16[:, 1:4, js].unsqueeze(2).broadcast_to((P, 3, 7, JH))
        c2b = c16[:, 2:5, js].unsqueeze(2).broadcast_to((P, 3, 7, JH))
        V.tensor_tensor(out=scr1[:, :, :, js], in0=c1b, in1=outE[:, 5:8, :, js],
                        op=OP.mult)
        V.tensor_tensor(out=scr2[:, :, :, js], in0=c2b, in1=outE[:, 4:7, :, js],
                        op=OP.mult)
        V.tensor_tensor(out=scr1[:, :, :, js], in0=scr1[:, :, :, js],
                        in1=scr2[:, :, :, js], op=OP.subtract)
        V.tensor_tensor(out=outE[:, 0:3, :, js], in0=scr1[:, :, :, js],
                        in1=jsm[:, :, :, js], op=OP.add)
        nc.sync.dma_start(out=dram["out"][:, 0:3, :, js], in_=outE[:, 0:3, :, js])


@functools.lru_cache(maxsize=1)
def _program():
    from contextlib import ExitStack
    import concourse.bacc as bacc
    import concourse.tile as tile
    from concourse import mybir

    f16 = mybir.dt.float16
    nc = bacc.Bacc("TRN2", target_bir_lowering=False, debug=False)
    dram = {
        "pc": nc.dram_tensor("pc", [P, 6, 7, J], f16, kind="ExternalInput"),
        "rot3": nc.dram_tensor("rot3", [P, 3, 7, J], f16, kind="ExternalInput"),
        "aux": nc.dram_tensor("aux", [P, 5, 7, J], f16, kind="ExternalInput"),
        "out": nc.dram_tensor("out", [P, 6, 7, J], f16, kind="ExternalOutput"),
    }
    with tile.TileContext(nc) as tc:
        with ExitStack() as ctx:
            _emit(nc, tc, ctx, dram)
    nc.compile()
    return nc


def _prep(com_list, link_pose_list):
    """Host-side packing: gather rot, fp16-cast, plane layout."""
    n = N_CORES * P * J
    com = np.ascontiguousarray(com_list, dtype=np.float32).reshape(n, 3, 7)
    pose = np.ascontiguousarray(link_pose_list, dtype=np.float32).reshape(n, 4, 4, 9)
    rot = pose[:, :3, AXIS, LINK] * SIGN                 # (n,3,7)
    drot = rot * (D_SUF.T * SC).astype(np.float32)
    pos = pose[:, :3, 3, :N_ACT]

    def pk(x, r):
        x16 = x.astype(np.float16).reshape(N_CORES, P, J, r, 7)
        return np.ascontiguousarray(x16.transpose(0, 1, 3, 4, 2))

    pcp = pk(np.concatenate([pos, com], axis=1), 6)
    rotp = pk(rot, 3)
    smrow = np.broadcast_to((SM * SC).astype(np.float32), (n, 1, 7))
    smtrow = np.broadcast_to((-SM / TM).astype(np.float32), (n, 1, 7))
    auxp = pk(np.concatenate([drot, smrow, smtrow], axis=1), 5)
    return [{"pc": pcp[k], "rot3": rotp[k], "aux": auxp[k]}
            for k in range(N_CORES)]


def _post(res):
    out = np.stack([res.results[k]["out"] for k in range(N_CORES)])  # (8,P,6,7,J)
    out = out.astype(np.float32).transpose(0, 1, 4, 2, 3)            # (8,P,J,6,7)
    return np.ascontiguousarray(out.reshape(512, 256, 6, 7))


def _kernel_bm0(com, pose):
    # bm=0 path (not exercised by the shipped setup_inputs; numpy fallback)
    rot = pose[:, :, :3, 2, :N_ACT].copy()
    rot[..., 1] = pose[:, :, :3, 0, 1]
    rot[..., 5] = pose[:, :, :3, 0, 5]
    rot[..., 4] *= -1.0
    delp = pose[:, :, :3, 3, -2][..., None] - pose[:, :, :3, 3, :N_ACT]
    jt = np.cross(rot, delp, axis=2)
    return np.concatenate([jt, rot], axis=2).astype(np.float32)


def kernel(com_list, link_pose_list, bm):
    com_list = np.ascontiguousarray(com_list, dtype=np.float32)
    link_pose_list = np.ascontiguousarray(link_pose_list, dtype=np.float32)
    if not int(bm):
        return _kernel_bm0(com_list, link_pose_list)

    from concourse.bass_utils import run_bass_kernel_spmd

    nc = _program()
    in_maps = _prep(com_list, link_pose_list)
    res = run_bass_kernel_spmd(nc, in_maps, core_ids=list(range(N_CORES)))
    return _post(res)


# revision 14
# speedup vs baseline: 2.2568x; 2.2568x over previous
"""Trainium2 Bass kernel for nn_CanadarmJacob (space-arm Jacobian, bm=1 path).

Contract: kernel(**inputs) takes FULL inputs (com_list (512,256,3,7) f32,
link_pose_list (512,256,4,4,9) f32, bm scalar) and returns the FULL output
(512,256,6,7) f32. Internally shards samples across 8 NeuronCores (pure data
parallel), runs a Bass/Tile kernel per core, and gathers.

Design (v4): fp16 datapath (DVE 2x_1p mode on tensor_tensor), plane layout
(P=128 partitions x comp x J=128 samples, J innermost/contiguous). Mass and
inertia constants pre-scaled by SC=1/64 so intermediates stay inside fp16
range; the scale cancels in bot = A @ Htheta because A inverts the scaled
H_s. Engine split: DVE runs the big fp16 tensor_tensor streams plus the
diagonal half of the fp32 3x3-inverse chain; ScalarE (ACT) runs the per-act
constant scalings (immediate-scale activations) plus copies/casts; GpSimd
runs the off-diagonal half of the inverse chain with memset-materialized
constants (its tensor_scalar ucode is pathologically slow, tensor_tensor is
fine). Output is written bot-first so its DMA overlaps the top computation.
"""
import sys
import functools

if "/opt/trn_rl_repo" not in sys.path:
    sys.path.insert(0, "/opt/trn_rl_repo")

import numpy as np

# ---------------------------------------------------------------- constants
N_CORES = 8
P = 128          # SBUF partitions
J = 128          # samples per partition per core
N_ACT = 7
SC = 1.0 / 64.0  # range scale for fp16

MASS = np.array([105.98, 105.98, 314.98, 279.2, 105.98, 105.98, 243.66], np.float64)
TM = float(MASS.sum() + 100000.0 + 243.66)
DIAGS = np.array([[12.19, 12.19, 3.061], [12.19, 12.19, 3.061], [15.41, 2094.71, 2103.19],
                  [9.522, 1966.28, 1966.28], [8.305, 3.061, 8.0386], [12.13, 12.13, 3.061],
                  [9.336, 44.41, 44.41]], np.float64)
D_SUF = np.cumsum(DIAGS[::-1], axis=0)[::-1]          # (7,3) suffix inertia diag
SM = np.cumsum(MASS[::-1])[::-1]                      # (7,) suffix mass
CD = DIAGS.sum(axis=0)                                # (3,)
_TF0 = np.array([[1, 0, 0, 0], [0, -1, 0, 0], [0, 0, 1.3, 6], [0, 0, 0, 1]], np.float64)
_COM0 = np.array([[1, 0, 0, 0], [0, 1, 0, 0], [0, 0, 1, 0.5], [0, 0, 0, 1]], np.float64)
BASE = (_TF0 @ _COM0)[:3, 3] * 243.66 / (100000.0 + 243.66)   # [0, 0, ~0.0162]

TMS = TM * SC
CDS = CD * SC
MS = MASS * SC
AXIS = np.array([2, 0, 2, 2, 2, 0, 2])
LINK = np.arange(N_ACT)
SIGN = np.array([1., 1., 1., 1., -1., 1., 1.], np.float32)


def _emit(nc, tc, ctx, dram):
    from concourse import mybir

    f16 = mybir.dt.float16
    f32 = mybir.dt.float32
    OP = mybir.AluOpType
    V = nc.vector
    G = nc.gpsimd
    S = nc.scalar

    pool = ctx.enter_context(tc.tile_pool(name="main", bufs=1))

    # fp16 tiles; *E tiles carry c-plane wrap-around extension [x,y,z,x,y]
    pc = pool.tile([P, 6, 7, J], f16)       # pos(0:3) | com(3:6)
    rot = pool.tile([P, 5, 7, J], f16)
    aux = pool.tile([P, 4, 7, J], f16)      # drot(0:3) | SM*SC plane
    delE = pool.tile([P, 5, 7, J], f16)
    md0 = pool.tile([P, 3, 7, J], f16)      # M*SC . del (ACT-written)
    wt = pool.tile([P, 3, 7, J], f16)       # suffix cumsum of md0 (DVE-written)
    scr1 = pool.tile([P, 3, 7, J], f16)
    scr2 = pool.tile([P, 3, 7, J], f16)
    jacE = pool.tile([P, 5, 7, J], f16)
    prod = pool.tile([P, 9, 7, J], f16)     # Sdiag | Soff | mcom products
    tscr = pool.tile([P, 9, 3, J], f16)
    red16 = pool.tile([P, 9, J], f16)
    vscr = pool.tile([P, 9, J], f16)
    smc = pool.tile([P, 3, 7, J], f16)
    w2E = pool.tile([P, 5, 7, J], f16)
    jsm = pool.tile([P, 3, 7, J], f16)
    hthE = pool.tile([P, 5, 7, J], f16)
    outE = pool.tile([P, 8, 7, J], f16)     # top(0:3) | bot(3:6) | bot-ext(6:8)
    c16 = pool.tile([P, 5, J], f16)
    A16 = pool.tile([P, 8, J], f16)         # a11,a22,a33,a12,a23,a13,a12,a23
    # fp32 smalls — each tile has a single engine writer (race safety)
    red32 = pool.tile([P, 9, J], f32)       # Sd(xx,yy,zz) | So(xy,yz,zx) | scom
    c32 = pool.tile([P, 3, J], f32)         # DVE
    sm32 = pool.tile([P, 8, J], f32)        # DVE rows: 0=SS 1=q 2=csq 3..7 scratch
    gp32 = pool.tile([P, 6, J], f32)        # GpSimd scratch: 0=TMS 1..5 scratch
    hsv = pool.tile([P, 3, J], f32)         # hxx,hyy,hzz (DVE)
    hsg = pool.tile([P, 3, J], f32)         # hxy,hyz,hzx (GpSimd)
    adjV = pool.tile([P, 2, J], f32)        # a11,a22 (DVE)
    adjG = pool.tile([P, 4, J], f32)        # a33,a12,a23,a13 (GpSimd)
    A32 = pool.tile([P, 6, J], f32)         # a11,a22,a33,a12,a23,a13 (DVE)

    nc.sync.dma_start(out=pc[:], in_=dram["pc"][:])
    nc.sync.dma_start(out=rot[:, 0:3], in_=dram["rot3"][:])
    nc.sync.dma_start(out=aux[:], in_=dram["aux"][:])

    def bc_c(ap):   # (P,7,J) -> (P,3,7,J), broadcast over coord planes
        return ap.unsqueeze(1).broadcast_to((P, 3, 7, J))

    def bc_a(ap, jn=J):   # (P,3,jn) -> (P,3,7,jn), broadcast over act
        return ap.unsqueeze(2).broadcast_to((P, 3, 7, jn))

    # --- del; per-act constant scalings on ACT ---------------------------
    V.tensor_tensor(out=delE[:, 0:3], in0=pc[:, 3:6], in1=pc[:, 0:3], op=OP.subtract)
    S.copy(out=delE[:, 3:5], in_=delE[:, 0:2])
    for a in range(N_ACT):
        S.mul(out=md0[:, :, a], in_=delE[:, 0:3, a], mul=float(MS[a]))
    for a in range(N_ACT):
        S.mul(out=prod[:, 6:9, a], in_=pc[:, 3:6, a], mul=float(MS[a]))

    # --- jac cross -------------------------------------------------------
    V.tensor_copy(out=rot[:, 3:5], in_=rot[:, 0:2])
    V.tensor_tensor(out=scr1[:], in0=rot[:, 1:4], in1=delE[:, 2:5], op=OP.mult)
    V.tensor_tensor(out=scr2[:], in0=rot[:, 2:5], in1=delE[:, 1:4], op=OP.mult)
    V.tensor_tensor(out=jacE[:, 0:3], in0=scr1[:], in1=scr2[:], op=OP.subtract)
    S.copy(out=jacE[:, 3:5], in_=jacE[:, 0:2])

    # --- S products, suffix cumsum, act-reduction tree -------------------
    V.tensor_tensor(out=prod[:, 0:3], in0=md0[:], in1=delE[:, 0:3], op=OP.mult)
    V.tensor_tensor(out=prod[:, 3:6], in0=md0[:], in1=delE[:, 1:4], op=OP.mult)
    V.tensor_copy(out=wt[:, :, 6], in_=md0[:, :, 6])
    for k in range(5, -1, -1):
        V.tensor_tensor(out=wt[:, :, k], in0=md0[:, :, k], in1=wt[:, :, k + 1],
                        op=OP.add)
    V.tensor_tensor(out=tscr[:], in0=prod[:, :, 0:3], in1=prod[:, :, 3:6], op=OP.add)
    V.tensor_tensor(out=red16[:], in0=tscr[:, :, 0], in1=tscr[:, :, 1], op=OP.add)
    V.tensor_tensor(out=vscr[:], in0=tscr[:, :, 2], in1=prod[:, :, 6], op=OP.add)
    V.tensor_tensor(out=red16[:], in0=red16[:], in1=vscr[:], op=OP.add)
    V.tensor_copy(out=red32[:], in_=red16[:])

    # --- c (ACT), then fp32 smalls split DVE/GpSimd ----------------------
    inv_tms = 1.0 / TMS
    V.tensor_scalar(out=c32[:, 0:2], in0=red32[:, 6:8], scalar1=inv_tms,
                    scalar2=None, op0=OP.mult)
    V.tensor_scalar(out=c32[:, 2], in0=red32[:, 8], scalar1=inv_tms,
                    scalar2=float(BASE[2]), op0=OP.mult, op1=OP.subtract)
    S.copy(out=c16[:, 0:3], in_=c32[:])
    S.copy(out=c16[:, 3:5], in_=c16[:, 0:2])

    # GpSimd branch: cc_off, hs_off (hsg = [hxy,hyz,hzx]), a33, (a23,a13), a12
    G.memset(gp32[:, 0], TMS)
    G.tensor_tensor(out=gp32[:, 1:3], in0=c32[:, 0:2], in1=c32[:, 1:3], op=OP.mult)
    G.tensor_tensor(out=gp32[:, 3], in0=c32[:, 2], in1=c32[:, 0], op=OP.mult)
    tmsb = gp32[:, 0].unsqueeze(1).broadcast_to((P, 3, J))
    G.tensor_tensor(out=gp32[:, 1:4], in0=gp32[:, 1:4], in1=tmsb, op=OP.mult)
    G.tensor_tensor(out=hsg[:], in0=gp32[:, 1:4], in1=red32[:, 3:6], op=OP.subtract)

    # DVE branch: SS, cc_diag, csq, q, hs_diag, (a11,a22)
    V.tensor_tensor(out=sm32[:, 0], in0=red32[:, 0], in1=red32[:, 1], op=OP.add)
    V.tensor_tensor(out=sm32[:, 0], in0=sm32[:, 0], in1=red32[:, 2], op=OP.add)
    ccd = sm32[:, 5:8]
    V.tensor_tensor(out=ccd, in0=c32[:, 0:3], in1=c32[:, 0:3], op=OP.mult)
    V.tensor_tensor(out=sm32[:, 2], in0=sm32[:, 5], in1=sm32[:, 6], op=OP.add)
    V.tensor_tensor(out=sm32[:, 2], in0=sm32[:, 2], in1=sm32[:, 7], op=OP.add)
    V.tensor_scalar(out=sm32[:, 1], in0=sm32[:, 2], scalar1=TMS, scalar2=None,
                    op0=OP.mult)
    V.tensor_tensor(out=sm32[:, 1], in0=sm32[:, 0], in1=sm32[:, 1], op=OP.subtract)
    for cx in range(3):
        V.tensor_scalar(out=hsv[:, cx], in0=sm32[:, 5 + cx], scalar1=TMS,
                        scalar2=float(CDS[cx]), op0=OP.mult, op1=OP.add)
    qb = sm32[:, 1].unsqueeze(1).broadcast_to((P, 3, J))
    V.tensor_tensor(out=sm32[:, 3:6], in0=qb, in1=red32[:, 0:3], op=OP.subtract)
    V.tensor_tensor(out=hsv[:], in0=hsv[:], in1=sm32[:, 3:6], op=OP.add)
    h2b = hsv[:, 2].unsqueeze(1).broadcast_to((P, 2, J))
    V.tensor_tensor(out=sm32[:, 3:5], in0=hsv[:, 1::-1], in1=h2b, op=OP.mult)
    V.tensor_tensor(out=sm32[:, 5:7], in0=hsg[:, 1:3], in1=hsg[:, 1:3], op=OP.mult)
    V.tensor_tensor(out=adjV[:], in0=sm32[:, 3:5], in1=sm32[:, 5:7], op=OP.subtract)

    # GpSimd: adjG rows [a33, a12, a23, a13] (emitted after hsv/hsg are written)
    G.tensor_tensor(out=gp32[:, 1], in0=hsv[:, 0], in1=hsv[:, 1], op=OP.mult)
    G.tensor_tensor(out=gp32[:, 2], in0=hsg[:, 0], in1=hsg[:, 0], op=OP.mult)
    G.tensor_tensor(out=adjG[:, 0], in0=gp32[:, 1], in1=gp32[:, 2], op=OP.subtract)
    h3b = hsg[:, 0].unsqueeze(1).broadcast_to((P, 2, J))
    G.tensor_tensor(out=gp32[:, 1:3], in0=h3b, in1=hsg[:, 2:0:-1], op=OP.mult)
    G.tensor_tensor(out=gp32[:, 3:5], in0=hsv[:, 0:2], in1=hsg[:, 1:3], op=OP.mult)
    G.tensor_tensor(out=adjG[:, 2:4], in0=gp32[:, 1:3], in1=gp32[:, 3:5],
                    op=OP.subtract)
    G.tensor_tensor(out=gp32[:, 1], in0=hsg[:, 1], in1=hsg[:, 2], op=OP.mult)
    G.tensor_tensor(out=gp32[:, 2], in0=hsg[:, 0], in1=hsv[:, 2], op=OP.mult)
    G.tensor_tensor(out=adjG[:, 1], in0=gp32[:, 1], in1=gp32[:, 2], op=OP.subtract)

    # --- w2, jsm (ACT), Htheta ------------------------------------------
    V.tensor_tensor(out=smc[:], in0=bc_c(aux[:, 3]), in1=bc_a(c16[:, 0:3]), op=OP.mult)
    V.tensor_tensor(out=w2E[:, 0:3], in0=wt[:], in1=smc[:], op=OP.subtract)
    V.tensor_copy(out=w2E[:, 3:5], in_=w2E[:, 0:2])
    for a in range(N_ACT):
        S.mul(out=jsm[:, :, a], in_=jacE[:, 0:3, a], mul=float(-SM[a] / TM))
    V.tensor_tensor(out=scr1[:], in0=w2E[:, 1:4], in1=jacE[:, 2:5], op=OP.mult)
    V.tensor_tensor(out=scr2[:], in0=w2E[:, 2:5], in1=jacE[:, 1:4], op=OP.mult)
    V.tensor_tensor(out=hthE[:, 0:3], in0=scr1[:], in1=scr2[:], op=OP.subtract)
    V.tensor_tensor(out=hthE[:, 0:3], in0=hthE[:, 0:3], in1=aux[:, 0:3], op=OP.add)
    V.tensor_copy(out=hthE[:, 3:5], in_=hthE[:, 0:2])

    # --- det, -1/det, A --------------------------------------------------
    # det = h0*a11 + h3*a12 + h5*a13
    V.tensor_tensor(out=sm32[:, 3], in0=hsv[:, 0], in1=adjV[:, 0], op=OP.mult)
    V.tensor_tensor(out=sm32[:, 4], in0=hsg[:, 0], in1=adjG[:, 1], op=OP.mult)
    V.tensor_tensor(out=sm32[:, 5], in0=hsg[:, 2], in1=adjG[:, 3], op=OP.mult)
    V.tensor_tensor(out=sm32[:, 6], in0=sm32[:, 3], in1=sm32[:, 4], op=OP.add)
    V.tensor_tensor(out=sm32[:, 6], in0=sm32[:, 6], in1=sm32[:, 5], op=OP.add)
    V.reciprocal(out=sm32[:, 7], in_=sm32[:, 6])
    rd2b = sm32[:, 7].unsqueeze(1).broadcast_to((P, 2, J))
    rd4b = sm32[:, 7].unsqueeze(1).broadcast_to((P, 4, J))
    V.tensor_tensor(out=A32[:, 0:2], in0=adjV[:], in1=rd2b, op=OP.mult)
    V.tensor_tensor(out=A32[:, 2:6], in0=adjG[:], in1=rd4b, op=OP.mult)
    S.mul(out=A16[:, 0:6], in_=A32[:], mul=-1.0)
    S.copy(out=A16[:, 6:8], in_=A16[:, 3:5])

    # --- bot = A @ Hth ---------------------------------------------------
    def Ab(r):
        return A16[:, r:r + 3].unsqueeze(2).broadcast_to((P, 3, 7, J))

    V.tensor_tensor(out=scr1[:], in0=Ab(0), in1=hthE[:, 0:3], op=OP.mult)
    V.tensor_tensor(out=scr2[:], in0=Ab(3), in1=hthE[:, 1:4], op=OP.mult)
    V.tensor_tensor(out=scr1[:], in0=scr1[:], in1=scr2[:], op=OP.add)
    V.tensor_tensor(out=scr2[:], in0=Ab(5), in1=hthE[:, 2:5], op=OP.mult)
    V.tensor_tensor(out=outE[:, 3:6], in0=scr1[:], in1=scr2[:], op=OP.add)
    V.tensor_copy(out=outE[:, 6:8], in_=outE[:, 3:5])
    nc.sync.dma_start(out=dram["out"][:, 3:6], in_=outE[:, 3:6])

    # --- top = c x bot + jsm; final add in J-halves to overlap out DMA ---
    V.tensor_tensor(out=scr1[:], in0=bc_a(c16[:, 1:4]), in1=outE[:, 5:8], op=OP.mult)
    V.tensor_tensor(out=scr2[:], in0=bc_a(c16[:, 2:5]), in1=outE[:, 4:7], op=OP.mult)
    V.tensor_tensor(out=scr1[:], in0=scr1[:], in1=scr2[:], op=OP.subtract)
    JH = J // 2
    for h in range(2):
        js = slice(h * JH, (h + 1) * JH)
        V.tensor_tensor(out=outE[:, 0:3, :, js], in0=scr1[:, :, :, js],
                        in1=jsm[:, :, :, js], op=OP.add)
        nc.sync.dma_start(out=dram["out"][:, 0:3, :, js], in_=outE[:, 0:3, :, js])


@functools.lru_cache(maxsize=1)
def _program():
    from contextlib import ExitStack
    import concourse.bacc as bacc
    import concourse.tile as tile
    from concourse import mybir

    f16 = mybir.dt.float16
    nc = bacc.Bacc("TRN2", target_bir_lowering=False, debug=False)
    dram = {
        "pc": nc.dram_tensor("pc", [P, 6, 7, J], f16, kind="ExternalInput"),
        "rot3": nc.dram_tensor("rot3", [P, 3, 7, J], f16, kind="ExternalInput"),
        "aux": nc.dram_tensor("aux", [P, 4, 7, J], f16, kind="ExternalInput"),
        "out": nc.dram_tensor("out", [P, 6, 7, J], f16, kind="ExternalOutput"),
    }
    with tile.TileContext(nc) as tc:
        with ExitStack() as ctx:
            _emit(nc, tc, ctx, dram)
    nc.compile()
    return nc


def _prep(com_list, link_pose_list):
    """Host-side packing: gather rot, fp16-cast, plane layout."""
    n = N_CORES * P * J
    com = np.ascontiguousarray(com_list, dtype=np.float32).reshape(n, 3, 7)
    pose = np.ascontiguousarray(link_pose_list, dtype=np.float32).reshape(n, 4, 4, 9)
    rot = pose[:, :3, AXIS, LINK] * SIGN                 # (n,3,7)
    drot = rot * (D_SUF.T * SC).astype(np.float32)
    pos = pose[:, :3, 3, :N_ACT]

    def pk(x, r):
        x16 = x.astype(np.float16).reshape(N_CORES, P, J, r, 7)
        return np.ascontiguousarray(x16.transpose(0, 1, 3, 4, 2))

    pcp = pk(np.concatenate([pos, com], axis=1), 6)
    rotp = pk(rot, 3)
    smrow = np.broadcast_to((SM * SC).astype(np.float32), (n, 1, 7))
    auxp = pk(np.concatenate([drot, smrow], axis=1), 4)
    return [{"pc": pcp[k], "rot3": rotp[k], "aux": auxp[k]}
            for k in range(N_CORES)]


def _post(res):
    out = np.stack([res.results[k]["out"] for k in range(N_CORES)])  # (8,P,6,7,J)
    out = out.astype(np.float32).transpose(0, 1, 4, 2, 3)            # (8,P,J,6,7)
    return np.ascontiguousarray(out.reshape(512, 256, 6, 7))


def _kernel_bm0(com, pose):
    # bm=0 path (not exercised by the shipped setup_inputs; numpy fallback)
    rot = pose[:, :, :3, 2, :N_ACT].copy()
    rot[..., 1] = pose[:, :, :3, 0, 1]
    rot[..., 5] = pose[:, :, :3, 0, 5]
    rot[..., 4] *= -1.0
    delp = pose[:, :, :3, 3, -2][..., None] - pose[:, :, :3, 3, :N_ACT]
    jt = np.cross(rot, delp, axis=2)
    return np.concatenate([jt, rot], axis=2).astype(np.float32)


def kernel(com_list, link_pose_list, bm):
    com_list = np.ascontiguousarray(com_list, dtype=np.float32)
    link_pose_list = np.ascontiguousarray(link_pose_list, dtype=np.float32)
    if not int(bm):
        return _kernel_bm0(com_list, link_pose_list)

    from concourse.bass_utils import run_bass_kernel_spmd

    nc = _program()
    in_maps = _prep(com_list, link_pose_list)
    res = run_bass_kernel_spmd(nc, in_maps, core_ids=list(range(N_CORES)))
    return _post(res)
